# revision 24
# baseline (speedup 1.0000x reference)
"""Multi-head self-attention (B=2, T=2048, D=1024, 16 heads) on 8 TRN2 cores.

Sharding: core c = (b, g) with b = c // 4 (batch), g = c % 4 (head group of 4).
Each core computes q/k/v projections for its 4 heads, causal softmax
attention, and a partial output projection (its 256 columns of the
concat-head dim against Wo). Host sums the 4 partials per batch and adds bo.

Fully-streamed schedule: the work is issued per query-block J (0..3) —
projection n-tiles for J, v chunks 4J..4J+3, then the two head-pair
attention blocks for J, with the previous J's output projection issued
as soon as its normalize lands. All phases share one instruction stream
so the Tile scheduler can fill Tensor-engine stalls (while the scalar
engine runs exp) with independent projection / output matmuls, keeping
the PE continuously busy (p-state at full clock).

Engine budget: PE ~93us of rows; scalar = exp only (+half the output
PSUM drains); DVE = projection/v drains with fused bias, normalize
copies + approx reciprocal; gpsimd = causal tri-masks + partition
broadcasts. PSUM: scores 4 banks + AV accumulators 2 + shared aux 2 = 8.

Per-block pipeline (unchanged math): scoresT chunks [tk=128, tq<=512] =
kT.T @ qT row-packed via tile_position (0,0)/(64,0); ACT exp(0.125 x)
PSUM->SBUF, one instr per chunk covering both heads; diagonal chunks
column-restricted to the causal region and tri-masked; AV accumulates
[v|1].T @ exp into at [65, 512] (row 64 = softmax denominators);
normalize via DVE reciprocal_approx_fast + gpsimd partition broadcast.
"""

import ml_dtypes
import numpy as np

import concourse.bass as bass
import concourse.tile as tile
from concourse import bacc, mybir
from concourse import bass_utils
from contextlib import ExitStack

F32 = mybir.dt.float32
BF16 = mybir.dt.bfloat16
ATT = BF16  # dtype for attention-phase matmul operands
AF = mybir.ActivationFunctionType
OP = mybir.AluOpType

B, T, D = 2, 2048, 1024
NH, DH = 16, 64
HPC = 4            # heads per core
GD = HPC * DH      # 256, group dim
GV = HPC * (DH + 1)  # 260, v tile width (64 v cols + ones col per head)
NKD = D // 128     # 8 K-chunks for projections
NT = T // 128      # 16 token chunks
NJ = T // 512      # 4 query blocks

_NC_CACHE = {}


def build():
    if "nc" in _NC_CACHE:
        return _NC_CACHE["nc"]
    nc = bacc.Bacc("TRN2", target_bir_lowering=False, debug=False, num_devices=8)

    # All bulk inputs are pre-swizzled host-side into [128, ...] partition
    # layout so each is a single contiguous DMA.
    HT = nc.dram_tensor("HT", [128, NKD * T], BF16, kind="ExternalInput").ap()
    WqT = nc.dram_tensor("WqT", [128, NKD * GD], BF16, kind="ExternalInput").ap()
    WkT = nc.dram_tensor("WkT", [128, NKD * GD], BF16, kind="ExternalInput").ap()
    WvS = nc.dram_tensor("WvS", [128, NKD * GV], BF16, kind="ExternalInput").ap()
    WoS = nc.dram_tensor("WoS", [128, 2 * D], BF16, kind="ExternalInput").ap()
    bqc = nc.dram_tensor("bqc", [128, 2], F32, kind="ExternalInput").ap()
    bkc = nc.dram_tensor("bkc", [128, 2], F32, kind="ExternalInput").ap()
    bvS = nc.dram_tensor("bvS", [1, GV], BF16, kind="ExternalInput").ap()
    kpm = nc.dram_tensor("kpm", [128, NT], F32, kind="ExternalInput").ap()
    O = nc.dram_tensor("O", [T, D], BF16, kind="ExternalOutput").ap()

    QS = [nc.sync, nc.scalar, nc.gpsimd]

    with tile.TileContext(nc) as tc, ExitStack() as octx:
        pool = octx.enter_context(tc.tile_pool(name="main", bufs=1))
        psum = octx.enter_context(tc.tile_pool(name="ps", bufs=1, space="PSUM"))

        # ---- constants (tiny DMAs first on sync) ----
        bq_sb = pool.tile([128, 2], F32, name="bq_sb", tag="bq_sb")
        bk_sb = pool.tile([128, 2], F32, name="bk_sb", tag="bk_sb")
        bv_r = pool.tile([1, GV], BF16, name="bv_r", tag="bv_r")
        kpm_sb = pool.tile([128, NT], F32, name="kpm_sb", tag="kpm_sb")
        nc.sync.dma_start(bq_sb[:], bqc[:])
        nc.sync.dma_start(bk_sb[:], bkc[:])
        nc.sync.dma_start(bv_r[:], bvS[:])
        nc.sync.dma_start(kpm_sb[:], kpm[:])

        ones_r = pool.tile([1, 128], BF16, name="ones_r", tag="ones_r")
        nc.vector.memset(ones_r[:], 1.0)

        # lower-tri mask [128,128]: keep where f >= p
        tri = pool.tile([128, 128], ATT, name="tri", tag="tri")
        nc.gpsimd.memset(tri[:], 1.0)
        nc.gpsimd.affine_select(
            out=tri[:], in_=tri[:], compare_op=OP.is_ge, fill=0.0,
            base=0, pattern=[[1, 128]], channel_multiplier=-1,
        )

        # ---- long-lived tiles ----
        ht_r = [pool.tile([128, T], BF16, name=f"ht{k}", tag=f"ht{k}") for k in range(NKD)]
        wq_r = pool.tile([128, NKD * GD], BF16, name="wq_r", tag="wq_r")
        wk_r = pool.tile([128, NKD * GD], BF16, name="wk_r", tag="wk_r")
        wv_r = pool.tile([128, NKD * GV], BF16, name="wv_r", tag="wv_r")
        wo_r = [pool.tile([128, D], BF16, name=f"wo{i}", tag=f"wo{i}") for i in range(2)]

        qT = [pool.tile([128, T], ATT, name=f"qT{m}", tag=f"qT{m}") for m in range(2)]
        kT = [pool.tile([128, T], ATT, name=f"kT{m}", tag=f"kT{m}") for m in range(2)]
        vt = [pool.tile([128, GV], ATT, name=f"vt{t}", tag=f"vt{t}") for t in range(NT)]
        attT = [pool.tile([128, T], ATT, name=f"attT{m}", tag=f"attT{m}") for m in range(2)]

        # ---- bulk input DMAs ----
        # HT lands n-tile-granular (each prep(J) needs only column block J
        # of every chunk), n-major so the first projections start ~6us in.
        # q/k weights in halves so the k-chase isn't gated on a full
        # transfer; wo last (first needed ~40us in).
        nc.scalar.dma_start(wq_r[:, 0:4 * GD], WqT[:, 0:4 * GD])
        nc.gpsimd.dma_start(wk_r[:, 0:4 * GD], WkT[:, 0:4 * GD])
        for n in range(NJ):
            for k in range(NKD):
                QS[(n + k) % 3].dma_start(
                    ht_r[k][:, n * 512:(n + 1) * 512],
                    HT[:, k * T + n * 512: k * T + (n + 1) * 512])
            if n == 0:
                nc.scalar.dma_start(wq_r[:, 4 * GD:], WqT[:, 4 * GD:])
                nc.gpsimd.dma_start(wk_r[:, 4 * GD:], WkT[:, 4 * GD:])
                nc.gpsimd.dma_start(wv_r[:], WvS[:])
        nc.sync.dma_start(wo_r[0][:], WoS[:, 0:D])
        nc.sync.dma_start(wo_r[1][:], WoS[:, D:2 * D])

        # ---- building blocks (generators: each yield ~1 PE matmul of work
        # so b_block can interleave them into Tensor-engine slack while the
        # scalar engine runs exp) ----
        def proj_group(w_r, m, dest, J, bias_col):
            # dest[m][:, J*512:(J+1)*512] = sum_k W_km.T @ HT_k + bias
            ps = psum.tile([128, 512], F32, name="aux", tag="aux", bufs=2)
            for k in range(NKD):
                nc.tensor.matmul(
                    ps[:],
                    w_r[:, k * GD + m * 128: k * GD + m * 128 + 128],
                    ht_r[k][:, J * 512:(J + 1) * 512],
                    start=(k == 0), stop=(k == NKD - 1),
                )
                yield
            nc.vector.tensor_scalar_add(
                dest[m][:, J * 512:(J + 1) * 512], ps[:], bias_col)
            yield

        def v_chunk(t):
            # vt[t][:, h*65 : h*65+65] = [v_h | ones] for the 4 heads
            vp = psum.tile([128, 512], F32, name="aux", tag="aux", bufs=2)
            for k in range(NKD):
                nc.tensor.matmul(
                    vp[:, 0:GV],
                    ht_r[k][:, t * 128:(t + 1) * 128],
                    wv_r[:, k * GV:(k + 1) * GV],
                    start=(k == 0), stop=False,
                )
                if k % 2 == 1:
                    yield
            nc.tensor.matmul(
                vp[:, 0:GV], ones_r[:], bv_r[:], start=False, stop=True)
            nc.vector.tensor_copy(vt[t][:], vp[:, 0:GV])
            nc.vector.tensor_scalar_mul(vt[t][:], vt[t][:], kpm_sb[:, t:t + 1])
            yield

        def issue_sc_exp(J, hp, kc):
            off = max(0, 128 * (kc - 4 * J))
            w = 512 - off
            # both heads in one 2-bank tile, each half bank-aligned
            sc = psum.tile([128, 1024], F32, name="sc", tag="sc", bufs=2)
            for hh in range(2):
                nc.tensor.matmul(
                    sc[:, hh * 512:hh * 512 + w],
                    kT[hp][hh * 64:(hh + 1) * 64, kc * 128:(kc + 1) * 128],
                    qT[hp][hh * 64:(hh + 1) * 64, J * 512 + off:(J + 1) * 512],
                    start=True, stop=True,
                    tile_position=(hh * 64, 0),
                )
            ex = pool.tile([128, 1024], ATT, name="ex", tag="ex", bufs=8)
            nc.scalar.activation(
                ex[:].rearrange("p (h c) -> p h c", c=512)[:, :, 0:w],
                sc[:].rearrange("p (h c) -> p h c", c=512)[:, :, 0:w],
                AF.Exp, scale=0.125,
            )
            if off or kc == 4 * J:
                for hh in range(2):
                    nc.vector.tensor_tensor(
                        ex[:, hh * 512:hh * 512 + 128],
                        ex[:, hh * 512:hh * 512 + 128],
                        tri[:],
                        op=OP.mult,
                    )
            return ex

        def normalize(hp, J, at):
            # softmax denominators: DVE approx reciprocal of the PSUM sums
            # row (row 64 of the AV accumulator), staged through SBUF
            # (custom-DVE ops misread PSUM). Keeps the scalar engine
            # exp-only — no ACT LUT-set reloads (1283ns each).
            aus = []
            for hh in range(2):
                au = pool.tile([64, 512], F32, name="au", tag="au", bufs=6)
                nc.vector.tensor_copy(au[:], at[hh][0:64, :])
                srow = pool.tile([1, 512], F32, name="srow", tag="srow", bufs=6)
                nc.vector.tensor_copy(srow[:], at[hh][64:65, :])
                aus.append((au, srow))
            for hh in range(2):
                au, srow = aus[hh]
                rc = pool.tile([1, 512], F32, name="rc", tag="rc", bufs=6)
                nc.vector.reciprocal_approx_fast(rc[:], srow[:])
                rb = pool.tile([64, 512], F32, name="rb", tag="rb", bufs=6)
                nc.gpsimd.partition_broadcast(rb[:], rc[:])
                nc.vector.tensor_tensor(
                    attT[hp][hh * 64:(hh + 1) * 64, J * 512:(J + 1) * 512],
                    au[:],
                    rb[:],
                    op=OP.mult,
                )

        def c_block(J):
            # output projection for token chunks 4J..4J+3 (needs both hps'
            # attT columns for J normalized)
            for t in range(4 * J, 4 * J + 4):
                ot = pool.tile([128, D], BF16, name="ot", tag="ot", bufs=3)
                for n in range(2):
                    op = psum.tile([128, 512], F32, name="aux", tag="aux", bufs=2)
                    for hp in range(2):
                        nc.tensor.matmul(
                            op[:],
                            attT[hp][:, t * 128:(t + 1) * 128],
                            wo_r[hp][:, n * 512:(n + 1) * 512],
                            start=(hp == 0), stop=(hp == 1),
                        )
                        yield
                    if n == 0:
                        nc.vector.tensor_copy(ot[:, 0:512], op[:])
                    else:
                        nc.scalar.copy(ot[:, 512:1024], op[:])
                [nc.sync, nc.gpsimd][t % 2].dma_start(
                    O[t * 128:(t + 1) * 128, :], ot[:])
                yield

        state = {"pending": None}
        fillers = []
        late = []  # PE-work reservoir drained after the last B block

        def drain(n):
            k = 0
            while fillers and k < n:
                try:
                    next(fillers[0])
                    k += 1
                except StopIteration:
                    fillers.pop(0)

        def run_now(gen):
            for _ in gen:
                pass

        def qk_chase(J):
            # first projection n-tile for both q/k m=0: k-outer so the
            # matmuls chase the HT chunk DMAs as they land
            ps_q = psum.tile([128, 512], F32, name="aux", tag="aux", bufs=2)
            ps_k = psum.tile([128, 512], F32, name="aux", tag="aux", bufs=2)
            for k in range(NKD):
                for w_r, ps in ((wq_r, ps_q), (wk_r, ps_k)):
                    nc.tensor.matmul(
                        ps[:],
                        w_r[:, k * GD: k * GD + 128],
                        ht_r[k][:, J * 512:(J + 1) * 512],
                        start=(k == 0), stop=(k == NKD - 1),
                    )
            nc.vector.tensor_scalar_add(
                qT[0][:, J * 512:(J + 1) * 512], ps_q[:], bq_sb[:, 0:1])
            nc.vector.tensor_scalar_add(
                kT[0][:, J * 512:(J + 1) * 512], ps_k[:], bk_sb[:, 0:1])

        def issue_av(at, J, hp, kc, ex, first, last):
            off = max(0, 128 * (kc - 4 * J))
            w = 512 - off
            for hh in range(2):
                h = 2 * hp + hh
                nc.tensor.matmul(
                    at[hh][0:65, off:512],
                    vt[kc][:, h * 65:(h + 1) * 65],
                    ex[:, hh * 512:hh * 512 + w],
                    start=first, stop=last,
                )

        def b_block(J, hp):
            at = [
                psum.tile([128, 512], F32, name=f"at{hh}", tag="av", bufs=2)
                for hh in range(2)
            ]
            # diagonal i=0 first (full width, opens PSUM accumulation),
            # then off-diagonals, then narrow diagonals
            kcs = [4 * J] + list(range(4 * J)) + [4 * J + i for i in (1, 2, 3)]
            prev = None
            for ti, kc in enumerate(kcs):
                ex = issue_sc_exp(J, hp, kc)
                if ti == 1 and state["pending"] is not None:
                    php, pJ, pat = state["pending"]
                    state["pending"] = None
                    normalize(php, pJ, pat)
                    if php == 1:
                        fillers.append(c_block(pJ))
                drain(3)
                if prev is not None:
                    issue_av(at, J, hp, prev[0], prev[1], prev[2] == 0, False)
                prev = (kc, ex, ti)
            issue_av(at, J, hp, prev[0], prev[1], prev[2] == 0, True)
            if state["pending"] is not None:
                php, pJ, pat = state["pending"]
                normalize(php, pJ, pat)
                if php == 1:
                    fillers.append(c_block(pJ))
            state["pending"] = (hp, J, at)

        # ---- the streamed schedule ----
        # prep(J) = projections' n-tile J for q/k (both head pairs) and v
        # token chunks 4J..4J+3. B-pair order [1,2,3,0]: prep(2)/prep(3)
        # and the C blocks arrive as PE filler for the ACT-heavy middle
        # pairs, and the tiny J=0 pair plus C(3)/C(0) form the endgame.
        def prep(J, chase=False):
            if chase:
                qk_chase(J)
            else:
                yield from proj_group(wq_r, 0, qT, J, bq_sb[:, 0:1])
                yield from proj_group(wk_r, 0, kT, J, bk_sb[:, 0:1])
            for t in range(4 * J, 4 * J + 4):
                yield from v_chunk(t)
            yield from proj_group(wq_r, 1, qT, J, bq_sb[:, 1:2])
            yield from proj_group(wk_r, 1, kT, J, bk_sb[:, 1:2])

        def force(gen):
            # issue-order correctness: everything a B block reads must be
            # ISSUED before the block's readers — finish this generator
            # (and all queued ahead of it, FIFO) before proceeding
            while gen in fillers:
                drain(64)

        # B-pair order [0,2,3,1]: exp work starts right after prep(0); each
        # pair's prep and the previous pairs' output projections interleave
        # as PE filler inside the ACT-paced chunk streams; the endgame is
        # only normalize(1,1) + C(1).
        run_now(prep(0, chase=True))
        g1, g2, g3 = prep(1), prep(2), prep(3)
        fillers.append(g1)
        fillers.append(g2)
        b_block(0, 0)
        b_block(0, 1)
        force(g2)
        fillers.append(g3)
        b_block(2, 0)
        b_block(2, 1)
        force(g3)
        b_block(3, 0)
        b_block(3, 1)
        b_block(1, 0)
        b_block(1, 1)
        php, pJ, pat = state["pending"]
        normalize(php, pJ, pat)
        while fillers:
            drain(64)
        run_now(c_block(pJ))

    nc.compile()
    _NC_CACHE["nc"] = nc
    return nc


def _swz(a):
    # [K*128, C] -> [128, K*C]: partition-major layout for one-shot DMA
    k = a.shape[0] // 128
    return np.ascontiguousarray(
        a.reshape(k, 128, a.shape[1]).transpose(1, 0, 2).reshape(128, -1))


def _prep_core_inputs(H, key_padding_mask, Wq, bq, Wk, bk, Wv, bv, Wo, bo):
    keep = 1.0 - np.asarray(key_padding_mask, dtype=np.float32)  # [B, T]
    bf = ml_dtypes.bfloat16
    in_maps = []
    HTs = [_swz(np.ascontiguousarray(H[b].T)).astype(bf) for b in range(B)]
    for c in range(8):
        b, g = divmod(c, 4)
        sl = slice(g * GD, (g + 1) * GD)
        WvT = Wv[sl].T  # [D, GD]
        WvS = np.zeros((D, GV), dtype=np.float32)
        bvS = np.zeros((1, GV), dtype=np.float32)
        for h in range(HPC):
            WvS[:, h * 65:h * 65 + 64] = WvT[:, h * 64:(h + 1) * 64]
            bvS[0, h * 65:h * 65 + 64] = bv[sl][h * 64:(h + 1) * 64]
            bvS[0, h * 65 + 64] = 1.0
        bqg = bq[sl]
        bkg = bk[sl]
        in_maps.append({
            "HT": HTs[b],
            "WqT": _swz(Wq[sl].T.copy()).astype(bf),
            "WkT": _swz(Wk[sl].T.copy()).astype(bf),
            "WvS": _swz(WvS).astype(bf),
            "WoS": _swz(np.ascontiguousarray(Wo[:, sl].T)).astype(bf),
            "bqc": np.stack([bqg[0:128], bqg[128:256]], axis=1).astype(np.float32),
            "bkc": np.stack([bkg[0:128], bkg[128:256]], axis=1).astype(np.float32),
            "bvS": bvS.astype(bf),
            "kpm": np.ascontiguousarray(keep[b].reshape(NT, 128).T),
        })
    return in_maps


def kernel(H, key_padding_mask, Wq, bq, Wk, bk, Wv, bv, Wo, bo, _run_kwargs=None):
    H = np.asarray(H, dtype=np.float32)
    Wq = np.asarray(Wq, dtype=np.float32)
    Wk = np.asarray(Wk, dtype=np.float32)
    Wv = np.asarray(Wv, dtype=np.float32)
    Wo = np.asarray(Wo, dtype=np.float32)
    bq = np.asarray(bq, dtype=np.float32)
    bk = np.asarray(bk, dtype=np.float32)
    bv = np.asarray(bv, dtype=np.float32)
    bo = np.asarray(bo, dtype=np.float32)

    nc = build()
    in_maps = _prep_core_inputs(H, key_padding_mask, Wq, bq, Wk, bk, Wv, bv, Wo, bo)
    res = bass_utils.run_bass_kernel_spmd(
        nc, in_maps, core_ids=list(range(8)), **(_run_kwargs or {})
    )
    out = np.zeros((B, T, D), dtype=np.float32)
    for c in range(8):
        out[c // 4] += res.results[c]["O"].astype(np.float32)
    out += bo
    if _run_kwargs:
        kernel.last_result = res
    return out
